# revision 1
# baseline (speedup 1.0000x reference)
"""MoE layer (top-2 of 8 experts, selection shared across tokens) on 8 TRN2 cores.

Math (faithful to the reference):
    gates = softmax(x @ W_gate + b_gate)          [N, 8]
    idx0  = top-2 expert indices of token 0       [2]
    s     = per-token top-2 gate VALUES (desc)    [N, 2]
    out   = s0 * (x @ W[A] + b[A]) + s1 * (x @ W[B] + b[B])

Strategy: gating + top-2 is 0.2% of the FLOPs -> computed on host.  The two
active expert matmuls (275 GFLOP) are data-parallel sharded over tokens across
8 cores; expert weights are replicated.  Matmuls run in fp16 (values are small,
so fp16 range is safe and its 10-bit mantissa keeps rel-err ~3e-4),
accumulating fp32 in PSUM.
"""

import contextlib
import ctypes
import functools
import os
import sys
import types

import numpy as np

import concourse.bass as bass
import concourse.mybir as mybir
import concourse.tile as tile
from concourse import bacc
from concourse import bass_utils as _bass_utils
from concourse.bass_utils import run_bass_kernel_spmd

# The A/B expert matmuls share the same stationary x-chunk, but walrus emits a
# (redundant) LDWEIGHTS before every MATMUL, and for 4-byte dtypes the LDW
# path (187ns + ~40ns handoff) gates the matmul cadence.  Walrus has a dedup
# pass for exactly this, hardcoded off in bir_verify_and_optimise.  Only
# useful for f32r kernels: with 16-bit operands the (FWL-style) LDWEIGHTS is
# rejected by that pass and is fully hidden anyway.
ENABLE_LDW_OPT = False
if ENABLE_LDW_OPT and not getattr(_bass_utils, "_ldw_opt_patch", False):
    _orig_run_command = _bass_utils.run_command

    def _run_command_ldw_opt(argv, **kwargs):
        argv = [
            "--enable-ldw-opt=true" if a == "--enable-ldw-opt=false" else a
            for a in argv
        ]
        return _orig_run_command(argv, **kwargs)

    _bass_utils.run_command = _run_command_ldw_opt
    _bass_utils._ldw_opt_patch = True

N_CORES = 8
N, D_IN, D_HID = 16384, 2048, 2048
NT = N // N_CORES            # tokens per core
KP = 128                     # contraction chunk = partition dim
KCH = D_IN // KP             # 16 K-chunks
NB = 512                     # output column block (1 PSUM bank of fp32)
NBLK = D_HID // NB           # 4 output blocks
TQ = 256                     # token slice per x-stream piece
NQ = NT // TQ                # 8 slices
MPQ = TQ // 128              # m-tiles per slice

F32 = mybir.dt.float32
F32R = mybir.dt.float32r
BF16 = mybir.dt.bfloat16

FP16 = mybir.dt.float16

# The PE streams one moving-operand column per cycle regardless of dtype, but
# 16-bit operands keep the (FWL) weight load fully hidden (97ns vs 187ns) and
# halve DMA.  fp16 (10 mantissa bits) beats bf16 ~8x on accuracy at identical
# speed, and the value ranges here (|x|<~6, |W|<~0.12) are safely inside
# fp16 range.  PSUM accumulates fp32; the per-token top-2 scores applied in
# the epilogue stay fp32.
W_DT = FP16
X_DT = FP16

# Filled by test harness inspection: last BassKernelResults from a run.
LAST_RESULT = None


@functools.lru_cache(maxsize=1)
def _build():
    nc = bacc.Bacc("TRN2", target_bir_lowering=False, debug=False)
    xT = nc.dram_tensor("xT", [D_IN, NT], X_DT, kind="ExternalInput")
    wa = nc.dram_tensor("wa", [D_IN, D_HID], W_DT, kind="ExternalInput")
    wb = nc.dram_tensor("wb", [D_IN, D_HID], W_DT, kind="ExternalInput")
    # bias pre-replicated across partitions on host: brep[p, e, o] = b_sel[e, o]
    brep = nc.dram_tensor("brep", [128, 2, D_HID], F32, kind="ExternalInput")
    # per-token scores pre-arranged on host, partition-major:
    # sC[p, m, s] = top2_score[m*128 + p, s]
    sC = nc.dram_tensor("sC", [128, NT // 128, 2], F32, kind="ExternalInput")
    out = nc.dram_tensor("out", [NT, D_HID], F32, kind="ExternalOutput")

    MULT = mybir.AluOpType.mult
    ADD = mybir.AluOpType.add

    with tile.TileContext(nc) as tc:
        with (
            tc.tile_pool(name="cst", bufs=1) as cst,
            tc.tile_pool(name="wp", bufs=2) as wp,
            tc.tile_pool(name="xp", bufs=3) as xp,
            tc.tile_pool(name="ep", bufs=2) as ep,
            tc.tile_pool(name="ps", bufs=3, space=bass.MemorySpace.PSUM) as ps,
        ):
            # constants ride the gpsimd (SWDGE) queue: it is otherwise idle at
            # t=0, so the bias matmul + epilogue unblock within a few us while
            # the two HWDGE queues (sync: W stream, scalar: x stream) fill.
            # constants go FIRST on the two fast HWDGE queues: the epilogue of
            # the very first psum group needs them, and the SWDGE queue crawls
            # on gather patterns (measured ~8us for a 2048-packet gather).
            sC_sb = cst.tile([128, NT // 128, 2], F32)
            nc.sync.dma_start(sC_sb[:], sC[:])
            brep_sb = cst.tile([128, 2, D_HID], F32)
            nc.sync.dma_start(brep_sb[:, 0, :], brep[:, 0, :])
            nc.scalar.dma_start(brep_sb[:, 1, :], brep[:, 1, :])

            # sync + scalar are pure DMA-issue queues (no compute on either, so
            # a dma_start blocked on a tile-slot semaphore never stalls math).
            # Both W and x are split across the two HWDGE queues to halve
            # arrival latency; the q==0 x-slice is emitted ahead of the W
            # block so a new nb-block never starts x-starved.
            def load_x(q):
                x_t = []
                for k in range(KCH):
                    t = xp.tile([KP, TQ], X_DT, tag=f"x{k}")
                    eng = nc.sync if k % 2 == 0 else nc.scalar
                    eng.dma_start(
                        t[:], xT[k * KP:(k + 1) * KP, q * TQ:(q + 1) * TQ]
                    )
                    x_t.append(t)
                return x_t

            def load_w(nb, k, e, wd, nb_sl):
                t = wp.tile([KP, NB], W_DT, tag=f"w{e}_{k}")
                eng = nc.sync if (k + e) % 2 == 0 else nc.scalar
                eng.dma_start(t[:], wd[k * KP:(k + 1) * KP, nb_sl])
                return t

            for nb in range(NBLK):
                nb_sl = bass.ts(nb, NB)
                w_t = {}
                if nb == 0:
                    # cold start: interleave x-slice-0 and W chunks k-major so
                    # the PE can begin the k-loop as soon as chunk 0 lands.
                    x_first = []
                    for k in range(KCH):
                        t = xp.tile([KP, TQ], X_DT, tag=f"x{k}")
                        eng = nc.sync if k % 2 == 0 else nc.scalar
                        eng.dma_start(t[:], xT[k * KP:(k + 1) * KP, 0:TQ])
                        x_first.append(t)
                        for e, wd in enumerate((wa, wb)):
                            w_t[e, k] = load_w(nb, k, e, wd, nb_sl)
                else:
                    x_first = load_x(0)
                    for k in range(KCH):
                        for e, wd in enumerate((wa, wb)):
                            w_t[e, k] = load_w(nb, k, e, wd, nb_sl)
                for q in range(NQ):
                    x_t = x_first if q == 0 else load_x(q)
                    for mi in range(MPQ):
                        mg = q * MPQ + mi
                        pa = ps.tile([128, NB], F32, tag="pa")
                        pb = ps.tile([128, NB], F32, tag="pb")
                        for k in range(KCH):
                            xk = x_t[k][:, bass.ts(mi, 128)]
                            nc.tensor.matmul(
                                pa[:], xk, w_t[0, k][:],
                                start=(k == 0), stop=(k == KCH - 1),
                            )
                            nc.tensor.matmul(
                                pb[:], xk, w_t[1, k][:],
                                start=(k == 0), stop=(k == KCH - 1),
                            )
                        s0 = sC_sb[:, mg, 0:1]
                        s1 = sC_sb[:, mg, 1:2]
                        # epilogue on DVE: out = s0*(pa+bA) + s1*(pb+bB)
                        # (each op reads at most one PSUM input)
                        u = ep.tile([128, NB], F32, tag="u")
                        nc.vector.tensor_add(u[:], pa[:], brep_sb[:, 0, nb_sl])
                        t1 = ep.tile([128, NB], F32, tag="t1")
                        nc.vector.tensor_scalar_mul(t1[:], u[:], s0)
                        v = ep.tile([128, NB], F32, tag="v")
                        nc.vector.tensor_add(v[:], pb[:], brep_sb[:, 1, nb_sl])
                        o = ep.tile([128, NB], F32, tag="o")
                        nc.vector.scalar_tensor_tensor(
                            o[:], v[:], s1, t1[:], op0=MULT, op1=ADD
                        )
                        nc.gpsimd.dma_start(out[bass.ts(mg, 128), nb_sl], o[:])

    nc.compile()
    return nc


def _host_gating(x, W_gate, b_gate):
    logits = x @ W_gate + b_gate                       # [N, 8] fp32
    m = logits.max(axis=1, keepdims=True)
    e = np.exp(logits - m)
    gates = e / e.sum(axis=1, keepdims=True)
    idx0 = np.argsort(-gates[0], kind="stable")[:2]    # token-0 top-2 experts
    scores = -np.sort(-gates, axis=1)[:, :2]           # per-token top-2 values
    return idx0, np.ascontiguousarray(scores)


def kernel(x, W_experts, b_experts, W_gate, b_gate):
    global LAST_RESULT
    x = np.ascontiguousarray(np.asarray(x, dtype=np.float32))
    W_experts = np.asarray(W_experts, dtype=np.float32)
    b_experts = np.asarray(b_experts, dtype=np.float32)
    W_gate = np.asarray(W_gate, dtype=np.float32)
    b_gate = np.asarray(b_gate, dtype=np.float32)

    idx0, scores = _host_gating(x, W_gate, b_gate)
    w_np_dt = mybir.dt.np(W_DT)
    x_np_dt = mybir.dt.np(X_DT)
    wa = np.ascontiguousarray(W_experts[idx0[0]]).astype(w_np_dt)  # [D_IN, D_HID]
    wb = np.ascontiguousarray(W_experts[idx0[1]]).astype(w_np_dt)
    brep = np.ascontiguousarray(
        np.broadcast_to(b_experts[idx0][None], (128, 2, D_HID))
    ).astype(np.float32)

    xT_full = np.ascontiguousarray(x.astype(x_np_dt).T)            # [D_IN, N]

    nc = _build()
    in_maps = []
    for c in range(N_CORES):
        sl = slice(c * NT, (c + 1) * NT)
        in_maps.append(
            {
                "xT": np.ascontiguousarray(xT_full[:, sl]),
                "wa": wa,
                "wb": wb,
                "brep": brep,
                "sC": np.ascontiguousarray(
                    scores[sl].reshape(NT // 128, 128, 2).transpose(1, 0, 2)
                ),
            }
        )

    res = run_bass_kernel_spmd(nc, in_maps, list(range(N_CORES)))
    LAST_RESULT = res
    return np.concatenate([r["out"] for r in res.results], axis=0)



# revision 2
# speedup vs baseline: 1.0163x; 1.0163x over previous
"""MoE layer (top-2 of 8 experts, selection shared across tokens) on 8 TRN2 cores.

Math (faithful to the reference):
    gates = softmax(x @ W_gate + b_gate)          [N, 8]
    idx0  = top-2 expert indices of token 0       [2]
    s     = per-token top-2 gate VALUES (desc)    [N, 2]
    out   = s0 * (x @ W[A] + b[A]) + s1 * (x @ W[B] + b[B])

Strategy: gating + top-2 is 0.2% of the FLOPs -> computed on host.  The two
active expert matmuls (275 GFLOP) are data-parallel sharded over tokens across
8 cores; expert weights are replicated.  Matmuls run in fp16 (values are small,
so fp16 range is safe and its 10-bit mantissa keeps rel-err ~3e-4),
accumulating fp32 in PSUM.

The PE stream (2048 MMs x 512 cols @ 2.4 GHz = 437 us) is the hard floor;
everything else is arranged to keep the PE saturated from ~10 us on:
  - x is resident in SBUF (loaded once, 8.4 MB fp16), W streams once per
    nb-block (no x re-streaming; total DMA-in 25 MB vs 50 MB).
  - DMA is coarse (1 MB tiles): dma_start issue costs ~0.65 us on the
    issuing engine, so hundreds of small chunk loads throttle both the
    cold start and the nb-block boundaries.
  - constants ride the otherwise-idle gpsimd (SWDGE) queue as plain
    contiguous copies; sync/scalar (HWDGE) queues carry only x + W.
  - outputs are written fp16 (host upcasts) on the gpsimd queue, keeping
    the HWDGE queues free for W prefetch and halving the output bytes.
"""

import functools

import numpy as np

import concourse.bass as bass
import concourse.mybir as mybir
import concourse.tile as tile
from concourse import bacc
from concourse.bass_utils import run_bass_kernel_spmd

N_CORES = 8
N, D_IN, D_HID = 16384, 2048, 2048
NT = N // N_CORES            # tokens per core
KP = 128                     # contraction chunk = partition dim
KCH = D_IN // KP             # 16 K-chunks
KH = 8                       # K-chunks per W half-tile (2 DMAs per expert/block)
NB = 512                     # output column block (1 PSUM bank of fp32)
NBLK = D_HID // NB           # 4 output blocks
TQ = 256                     # tokens per resident x tile
NQ = NT // TQ                # 8 x tiles
MPQ = TQ // 128              # m-tiles per x tile
MG = NT // 128               # 16 token groups per core

F32 = mybir.dt.float32
FP16 = mybir.dt.float16

W_DT = FP16
X_DT = FP16
O_DT = FP16

# Filled by test harness inspection: last BassKernelResults from a run.
LAST_RESULT = None


@functools.lru_cache(maxsize=1)
def _build():
    nc = bacc.Bacc("TRN2", target_bir_lowering=False, debug=False)
    xT = nc.dram_tensor("xT", [D_IN, NT], X_DT, kind="ExternalInput")
    wa = nc.dram_tensor("wa", [D_IN, D_HID], W_DT, kind="ExternalInput")
    wb = nc.dram_tensor("wb", [D_IN, D_HID], W_DT, kind="ExternalInput")
    # bias pre-replicated across partitions on host: brep[p, e, o] = b_sel[e, o]
    brep = nc.dram_tensor("brep", [128, 2, D_HID], FP16, kind="ExternalInput")
    # per-token scores pre-arranged on host, partition-major:
    # sC[p, m, s] = top2_score[m*128 + p, s]
    sC = nc.dram_tensor("sC", [128, MG, 2], F32, kind="ExternalInput")
    out = nc.dram_tensor("out", [NT, D_HID], O_DT, kind="ExternalOutput")

    MULT = mybir.AluOpType.mult
    ADD = mybir.AluOpType.add

    wdr = (wa, wb)

    with tile.TileContext(nc) as tc:
        with (
            tc.tile_pool(name="cst", bufs=1) as cst,
            tc.tile_pool(name="xp", bufs=1) as xp,
            tc.tile_pool(name="wp", bufs=2) as wp,
            tc.tile_pool(name="ep", bufs=2) as ep,
            tc.tile_pool(name="op", bufs=3) as op,
            tc.tile_pool(name="ps", bufs=3, space=bass.MemorySpace.PSUM) as ps,
        ):
            # constants on the gpsimd (SWDGE) queue: it is otherwise idle until
            # the first epilogue's output DMA (~25 us), and these are plain
            # contiguous copies (sC 16 KB + brep 1 MB land within ~5 us).
            sC_sb = cst.tile([128, MG, 2], F32)
            nc.gpsimd.dma_start(sC_sb[:], sC[:])
            brep_sb = cst.tile([128, 2, D_HID], FP16)
            nc.gpsimd.dma_start(brep_sb[:], brep[:])

            # sync + scalar are pure DMA-issue queues (no compute on either).
            # One issue per MB keeps the engines essentially idle.
            hw_eng = [nc.sync, nc.scalar]

            def load_x(q, eng):
                t = xp.tile([KP, KCH, TQ], X_DT, tag=f"xq{q}")
                eng.dma_start(
                    t[:],
                    xT[:, q * TQ:(q + 1) * TQ].rearrange("(k p) t -> p k t", p=KP),
                )
                return t

            def load_w(nb, e, h, eng):
                t = wp.tile([KP, KH, NB], W_DT, tag=f"w{e}_{h}")
                eng.dma_start(
                    t[:],
                    wdr[e][h * KH * KP:(h + 1) * KH * KP, bass.ts(nb, NB)]
                    .rearrange("(k p) n -> p k n", p=KP),
                )
                return t

            # Cold-start interleave: the first group's k-loop consumes
            # xq0 + all four W(nb=0) tiles; order them first on the queues.
            x_t = [None] * NQ
            x_t[0] = load_x(0, nc.sync)
            w_cur = {}
            w_cur[0, 0] = load_w(0, 0, 0, nc.scalar)   # pa k0-7
            w_cur[1, 0] = load_w(0, 1, 0, nc.sync)     # pb k0-7
            w_cur[0, 1] = load_w(0, 0, 1, nc.scalar)   # pa k8-15
            w_cur[1, 1] = load_w(0, 1, 1, nc.sync)     # pb k8-15
            for q in range(1, NQ):
                x_t[q] = load_x(q, hw_eng[q % 2])

            for nb in range(NBLK):
                nb_sl = bass.ts(nb, NB)
                w_next = {} if nb + 1 < NBLK else None
                for q in range(NQ):
                    # W(nb+1) prefetch, one 1 MB tile per even q: spreads the
                    # transfers across the block so no queue burst exceeds
                    # ~3 us, with a full block's lead time.
                    if w_next is not None and q % 2 == 0:
                        i = q // 2
                        e, h = i % 2, i // 2
                        w_next[e, h] = load_w(nb + 1, e, h, hw_eng[i % 2])
                    for mi in range(MPQ):
                        mg = q * MPQ + mi
                        m_sl = bass.ts(mi, 128)
                        pa = ps.tile([128, NB], F32, tag="pa")
                        pb = ps.tile([128, NB], F32, tag="pb")
                        # expert-sequential k-loops: pa finishes 16 MMs early,
                        # so its epilogue ops overlap pb's accumulation.
                        for k in range(KCH):
                            nc.tensor.matmul(
                                pa[:], x_t[q][:, k, m_sl],
                                w_cur[0, k // KH][:, k % KH, :],
                                start=(k == 0), stop=(k == KCH - 1),
                            )
                        for k in range(KCH):
                            nc.tensor.matmul(
                                pb[:], x_t[q][:, k, m_sl],
                                w_cur[1, k // KH][:, k % KH, :],
                                start=(k == 0), stop=(k == KCH - 1),
                            )
                        s0 = sC_sb[:, mg, 0:1]
                        s1 = sC_sb[:, mg, 1:2]
                        # epilogue on DVE: out = s0*(pa+bA) + s1*(pb+bB)
                        # (each op reads at most one PSUM input)
                        u = ep.tile([128, NB], F32, tag="u")
                        nc.vector.tensor_add(u[:], pa[:], brep_sb[:, 0, nb_sl])
                        t1 = ep.tile([128, NB], F32, tag="t1")
                        nc.vector.tensor_scalar_mul(t1[:], u[:], s0)
                        v = ep.tile([128, NB], F32, tag="v")
                        nc.vector.tensor_add(v[:], pb[:], brep_sb[:, 1, nb_sl])
                        o = op.tile([128, NB], O_DT, tag="o")
                        nc.vector.scalar_tensor_tensor(
                            o[:], v[:], s1, t1[:], op0=MULT, op1=ADD
                        )
                        nc.gpsimd.dma_start(out[bass.ts(mg, 128), nb_sl], o[:])
                w_cur = w_next

    nc.compile()
    return nc


def _host_gating(x, W_gate, b_gate):
    logits = x @ W_gate + b_gate                       # [N, 8] fp32
    m = logits.max(axis=1, keepdims=True)
    e = np.exp(logits - m)
    gates = e / e.sum(axis=1, keepdims=True)
    idx0 = np.argsort(-gates[0], kind="stable")[:2]    # token-0 top-2 experts
    scores = -np.sort(-gates, axis=1)[:, :2]           # per-token top-2 values
    return idx0, np.ascontiguousarray(scores)


def kernel(x, W_experts, b_experts, W_gate, b_gate):
    global LAST_RESULT
    x = np.ascontiguousarray(np.asarray(x, dtype=np.float32))
    W_experts = np.asarray(W_experts, dtype=np.float32)
    b_experts = np.asarray(b_experts, dtype=np.float32)
    W_gate = np.asarray(W_gate, dtype=np.float32)
    b_gate = np.asarray(b_gate, dtype=np.float32)

    idx0, scores = _host_gating(x, W_gate, b_gate)
    w_np_dt = mybir.dt.np(W_DT)
    x_np_dt = mybir.dt.np(X_DT)
    wa = np.ascontiguousarray(W_experts[idx0[0]]).astype(w_np_dt)  # [D_IN, D_HID]
    wb = np.ascontiguousarray(W_experts[idx0[1]]).astype(w_np_dt)
    brep = np.ascontiguousarray(
        np.broadcast_to(b_experts[idx0][None], (128, 2, D_HID))
    ).astype(np.float16)

    xT_full = np.ascontiguousarray(x.astype(x_np_dt).T)            # [D_IN, N]

    nc = _build()
    in_maps = []
    for c in range(N_CORES):
        sl = slice(c * NT, (c + 1) * NT)
        in_maps.append(
            {
                "xT": np.ascontiguousarray(xT_full[:, sl]),
                "wa": wa,
                "wb": wb,
                "brep": brep,
                "sC": np.ascontiguousarray(
                    scores[sl].reshape(MG, 128, 2).transpose(1, 0, 2)
                ),
            }
        )

    res = run_bass_kernel_spmd(nc, in_maps, list(range(N_CORES)))
    LAST_RESULT = res
    return np.concatenate(
        [r["out"] for r in res.results], axis=0
    ).astype(np.float32)


# revision 5
# speedup vs baseline: 1.0190x; 1.0027x over previous
"""MoE layer (top-2 of 8 experts, selection shared across tokens) on 8 TRN2 cores.

Math (faithful to the reference):
    gates = softmax(x @ W_gate + b_gate)          [N, 8]
    idx0  = top-2 expert indices of token 0       [2]
    s     = per-token top-2 gate VALUES (desc)    [N, 2]
    out   = s0 * (x @ W[A] + b[A]) + s1 * (x @ W[B] + b[B])

Strategy: gating + top-2 is 0.2% of the FLOPs -> computed on host.  The two
active expert matmuls (275 GFLOP) are data-parallel sharded over tokens across
8 cores; expert weights are replicated.  Matmuls run in fp16 (values are small,
so fp16 range is safe and its 10-bit mantissa keeps rel-err ~3e-4),
accumulating fp32 in PSUM.

The PE stream (2048 MMs x 512 cols @ 2.4 GHz = 437 us) is the hard floor;
everything else is arranged to keep the PE saturated from ~10 us on:
  - x is resident in SBUF (loaded once, 8.4 MB fp16), W streams once per
    nb-block (no x re-streaming; total DMA-in 25 MB vs 50 MB).
  - DMA is coarse (1 MB tiles): dma_start issue costs ~0.65 us on the
    issuing engine, so hundreds of small chunk loads throttle both the
    cold start and the nb-block boundaries.
  - constants ride the otherwise-idle gpsimd (SWDGE) queue as plain
    contiguous copies; sync/scalar (HWDGE) queues carry only x + W.
  - outputs are written fp16 (host upcasts) on the gpsimd queue, keeping
    the HWDGE queues free for W prefetch and halving the output bytes.
"""

import functools

import numpy as np

import concourse.bass as bass
import concourse.mybir as mybir
import concourse.tile as tile
from concourse import bacc
from concourse.bass_utils import run_bass_kernel_spmd

N_CORES = 8
N, D_IN, D_HID = 16384, 2048, 2048
NT = N // N_CORES            # tokens per core
KP = 128                     # contraction chunk = partition dim
KCH = D_IN // KP             # 16 K-chunks
KH = 8                       # K-chunks per W half-tile (2 DMAs per expert/block)
NB = 512                     # output column block (1 PSUM bank of fp32)
NBLK = D_HID // NB           # 4 output blocks
TQ = 256                     # tokens per resident x tile
NQ = NT // TQ                # 8 x tiles
MPQ = TQ // 128              # m-tiles per x tile
MG = NT // 128               # 16 token groups per core

F32 = mybir.dt.float32
FP16 = mybir.dt.float16

W_DT = FP16
X_DT = FP16
O_DT = FP16

# Filled by test harness inspection: last BassKernelResults from a run.
LAST_RESULT = None


@functools.lru_cache(maxsize=1)
def _build():
    nc = bacc.Bacc("TRN2", target_bir_lowering=False, debug=False)
    xT = nc.dram_tensor("xT", [D_IN, NT], X_DT, kind="ExternalInput")
    wa = nc.dram_tensor("wa", [D_IN, D_HID], W_DT, kind="ExternalInput")
    wb = nc.dram_tensor("wb", [D_IN, D_HID], W_DT, kind="ExternalInput")
    # bias pre-replicated across partitions on host: brep[p, e, o] = b_sel[e, o]
    brep = nc.dram_tensor("brep", [128, 2, D_HID], FP16, kind="ExternalInput")
    # per-token scores pre-arranged on host, partition-major:
    # sC[p, m, s] = top2_score[m*128 + p, s]
    sC = nc.dram_tensor("sC", [128, MG, 2], F32, kind="ExternalInput")
    out = nc.dram_tensor("out", [NT, D_HID], O_DT, kind="ExternalOutput")

    MULT = mybir.AluOpType.mult
    ADD = mybir.AluOpType.add

    wdr = (wa, wb)

    with tile.TileContext(nc) as tc:
        with (
            tc.tile_pool(name="cst", bufs=1) as cst,
            tc.tile_pool(name="xp", bufs=1) as xp,
            tc.tile_pool(name="wp", bufs=2) as wp,
            tc.tile_pool(name="ep", bufs=2) as ep,
            tc.tile_pool(name="op", bufs=3) as op,
            tc.tile_pool(name="ps", bufs=3, space=bass.MemorySpace.PSUM) as ps,
        ):
            # sync + scalar are pure DMA-issue queues (no compute on either).
            # One issue per MB keeps the engines essentially idle.
            hw_eng = [nc.sync, nc.scalar]

            # sC is 16 KB — in front of xq0 it delays the fill by <100 ns and
            # the first epilogue (t1 mul) needs it early.
            sC_sb = cst.tile([128, MG, 2], F32)
            nc.sync.dma_start(sC_sb[:], sC[:])

            def load_x(q, eng):
                t = xp.tile([KP, KCH, TQ], X_DT, tag=f"xq{q}")
                eng.dma_start(
                    t[:],
                    xT[:, q * TQ:(q + 1) * TQ].rearrange("(k p) t -> p k t", p=KP),
                )
                return t

            def load_w(nb, e, h, eng):
                t = wp.tile([KP, KH, NB], W_DT, tag=f"w{e}_{h}")
                eng.dma_start(
                    t[:],
                    wdr[e][h * KH * KP:(h + 1) * KH * KP, bass.ts(nb, NB)]
                    .rearrange("(k p) n -> p k n", p=KP),
                )
                return t

            # Cold-start interleave: the first group's k-loop consumes
            # xq0 + all four W(nb=0) tiles; order them first on the queues.
            x_t = [None] * NQ
            x_t[0] = load_x(0, nc.sync)
            w_cur = {}
            w_cur[0, 0] = load_w(0, 0, 0, nc.scalar)   # pa k0-7
            w_cur[1, 0] = load_w(0, 1, 0, nc.sync)     # pb k0-7
            w_cur[0, 1] = load_w(0, 0, 1, nc.scalar)   # pa k8-15
            w_cur[1, 1] = load_w(0, 1, 1, nc.sync)     # pb k8-15
            # brep (1 MB fp16) queues behind the critical 5 MB fill; it lands
            # right as the first epilogue's bias-add wants it, and the ps
            # pool's 3-group slack absorbs any residual skew.
            brep_sb = cst.tile([128, 2, D_HID], FP16)
            nc.scalar.dma_start(brep_sb[:], brep[:])
            for q in range(1, NQ):
                x_t[q] = load_x(q, hw_eng[q % 2])

            for nb in range(NBLK):
                nb_sl = bass.ts(nb, NB)
                w_next = {} if nb + 1 < NBLK else None
                for q in range(NQ):
                    # W(nb+1) prefetch, one 1 MB tile per even q: spreads the
                    # transfers across the block so no queue burst exceeds
                    # ~3 us, with a full block's lead time.
                    if w_next is not None and q % 2 == 0:
                        i = q // 2
                        e, h = i % 2, i // 2
                        w_next[e, h] = load_w(nb + 1, e, h, hw_eng[i % 2])
                    for mi in range(MPQ):
                        mg = q * MPQ + mi
                        m_sl = bass.ts(mi, 128)
                        pa = ps.tile([128, NB], F32, tag="pa")
                        pb = ps.tile([128, NB], F32, tag="pb")
                        # expert-sequential k-loops: pa finishes 16 MMs early,
                        # so its epilogue ops overlap pb's accumulation.
                        for k in range(KCH):
                            nc.tensor.matmul(
                                pa[:], x_t[q][:, k, m_sl],
                                w_cur[0, k // KH][:, k % KH, :],
                                start=(k == 0), stop=(k == KCH - 1),
                            )
                        for k in range(KCH):
                            nc.tensor.matmul(
                                pb[:], x_t[q][:, k, m_sl],
                                w_cur[1, k // KH][:, k % KH, :],
                                start=(k == 0), stop=(k == KCH - 1),
                            )
                        s0 = sC_sb[:, mg, 0:1]
                        s1 = sC_sb[:, mg, 1:2]
                        # epilogue on DVE: out = s0*(pa+bA) + s1*(pb+bB)
                        # (each op reads at most one PSUM input)
                        u = ep.tile([128, NB], F32, tag="u")
                        nc.vector.tensor_add(u[:], pa[:], brep_sb[:, 0, nb_sl])
                        t1 = ep.tile([128, NB], F32, tag="t1")
                        nc.vector.tensor_scalar_mul(t1[:], u[:], s0)
                        v = ep.tile([128, NB], F32, tag="v")
                        nc.vector.tensor_add(v[:], pb[:], brep_sb[:, 1, nb_sl])
                        o = op.tile([128, NB], O_DT, tag="o")
                        nc.vector.scalar_tensor_tensor(
                            o[:], v[:], s1, t1[:], op0=MULT, op1=ADD
                        )
                        # outputs ride the HWDGE queues: the SWDGE (gpsimd)
                        # end-of-kernel drain costs ~6.5 us even when the last
                        # transfer landed long before; HWDGE drains instantly.
                        hw_eng[mg % 2].dma_start(
                            out[bass.ts(mg, 128), nb_sl], o[:]
                        )
                w_cur = w_next

    nc.compile()
    return nc


def _host_gating(x, W_gate, b_gate):
    logits = x @ W_gate + b_gate                       # [N, 8] fp32
    m = logits.max(axis=1, keepdims=True)
    e = np.exp(logits - m)
    gates = e / e.sum(axis=1, keepdims=True)
    idx0 = np.argsort(-gates[0], kind="stable")[:2]    # token-0 top-2 experts
    scores = -np.sort(-gates, axis=1)[:, :2]           # per-token top-2 values
    return idx0, np.ascontiguousarray(scores)


def kernel(x, W_experts, b_experts, W_gate, b_gate):
    global LAST_RESULT
    x = np.ascontiguousarray(np.asarray(x, dtype=np.float32))
    W_experts = np.asarray(W_experts, dtype=np.float32)
    b_experts = np.asarray(b_experts, dtype=np.float32)
    W_gate = np.asarray(W_gate, dtype=np.float32)
    b_gate = np.asarray(b_gate, dtype=np.float32)

    idx0, scores = _host_gating(x, W_gate, b_gate)
    w_np_dt = mybir.dt.np(W_DT)
    x_np_dt = mybir.dt.np(X_DT)
    wa = np.ascontiguousarray(W_experts[idx0[0]]).astype(w_np_dt)  # [D_IN, D_HID]
    wb = np.ascontiguousarray(W_experts[idx0[1]]).astype(w_np_dt)
    brep = np.ascontiguousarray(
        np.broadcast_to(b_experts[idx0][None], (128, 2, D_HID))
    ).astype(np.float16)

    xT_full = np.ascontiguousarray(x.astype(x_np_dt).T)            # [D_IN, N]

    nc = _build()
    in_maps = []
    for c in range(N_CORES):
        sl = slice(c * NT, (c + 1) * NT)
        in_maps.append(
            {
                "xT": np.ascontiguousarray(xT_full[:, sl]),
                "wa": wa,
                "wb": wb,
                "brep": brep,
                "sC": np.ascontiguousarray(
                    scores[sl].reshape(MG, 128, 2).transpose(1, 0, 2)
                ),
            }
        )

    res = run_bass_kernel_spmd(nc, in_maps, list(range(N_CORES)))
    LAST_RESULT = res
    return np.concatenate(
        [r["out"] for r in res.results], axis=0
    ).astype(np.float32)


# revision 10
# speedup vs baseline: 1.0220x; 1.0029x over previous
"""MoE layer (top-2 of 8 experts, selection shared across tokens) on 8 TRN2 cores.

Math (faithful to the reference):
    gates = softmax(x @ W_gate + b_gate)          [N, 8]
    idx0  = top-2 expert indices of token 0       [2]
    s     = per-token top-2 gate VALUES (desc)    [N, 2]
    out   = s0 * (x @ W[A] + b[A]) + s1 * (x @ W[B] + b[B])

Strategy: gating + top-2 is 0.2% of the FLOPs -> computed on host.  The two
active expert matmuls (275 GFLOP) are data-parallel sharded over tokens across
8 cores; expert weights are replicated.  Matmuls run in fp16 (values are small,
so fp16 range is safe and its 10-bit mantissa keeps rel-err ~3e-4),
accumulating fp32 in PSUM.

The PE stream (2048 MMs x 512 cols @ 2.4 GHz = 437 us) is the hard floor;
everything else is arranged to keep the PE saturated from ~10 us on:
  - x is resident in SBUF (loaded once, 8.4 MB fp16), W streams once per
    nb-block (no x re-streaming; total DMA-in 25 MB vs 50 MB).
  - DMA is coarse (1 MB tiles): dma_start issue costs ~0.65 us on the
    issuing engine, so hundreds of small chunk loads throttle both the
    cold start and the nb-block boundaries.
  - constants ride the otherwise-idle gpsimd (SWDGE) queue as plain
    contiguous copies; sync/scalar (HWDGE) queues carry only x + W.
  - outputs are written fp16 (host upcasts) on the gpsimd queue, keeping
    the HWDGE queues free for W prefetch and halving the output bytes.
"""

import functools

import numpy as np

import concourse.bass as bass
import concourse.mybir as mybir
import concourse.tile as tile
from concourse import bacc
from concourse.bass_utils import run_bass_kernel_spmd

N_CORES = 8
N, D_IN, D_HID = 16384, 2048, 2048
NT = N // N_CORES            # tokens per core
KP = 128                     # contraction chunk = partition dim
KCH = D_IN // KP             # 16 K-chunks
KH = 8                       # K-chunks per W half-tile (2 DMAs per expert/block)
NB = 512                     # output column block (1 PSUM bank of fp32)
NBLK = D_HID // NB           # 4 output blocks
TQ = 256                     # tokens per resident x tile
NQ = NT // TQ                # 8 x tiles
MPQ = TQ // 128              # m-tiles per x tile
MG = NT // 128               # 16 token groups per core

F32 = mybir.dt.float32
FP16 = mybir.dt.float16

W_DT = FP16
X_DT = FP16
O_DT = FP16

# Filled by test harness inspection: last BassKernelResults from a run.
LAST_RESULT = None


@functools.lru_cache(maxsize=1)
def _build():
    nc = bacc.Bacc("TRN2", target_bir_lowering=False, debug=False)
    # x and W are PRE-PACKED on host so every SBUF tile is contiguous per
    # partition (8 KB descriptors).  SDMA engines round-robin between the two
    # HWDGE queues at PACKET granularity and a packet is one descriptor run:
    # with 512 B x-descriptors vs 1-8 KB W-descriptors the x queue got ~1/3
    # of the bandwidth and the cold-start fill stretched by ~8 us.
    # xQ[q] -> one x tile [128, KCH, TQ]; wP[nb, e, h] -> one W tile
    # [128, KH, NB].
    xQ = nc.dram_tensor("xQ", [NQ, 128, KCH, TQ], X_DT, kind="ExternalInput")
    wP = nc.dram_tensor("wP", [NBLK, 2, 2, 128, KH, NB], W_DT,
                        kind="ExternalInput")
    # bias pre-replicated across partitions on host: brep[p, e, o] = b_sel[e, o]
    brep = nc.dram_tensor("brep", [128, 2, D_HID], FP16, kind="ExternalInput")
    # per-token scores pre-arranged on host, partition-major:
    # sC[p, m, s] = top2_score[m*128 + p, s]
    sC = nc.dram_tensor("sC", [128, MG, 2], F32, kind="ExternalInput")
    out = nc.dram_tensor("out", [NT, D_HID], O_DT, kind="ExternalOutput")

    MULT = mybir.AluOpType.mult
    ADD = mybir.AluOpType.add

    with tile.TileContext(nc) as tc:
        with (
            tc.tile_pool(name="cst", bufs=1) as cst,
            tc.tile_pool(name="xp", bufs=1) as xp,
            tc.tile_pool(name="wp", bufs=2) as wp,
            tc.tile_pool(name="ep", bufs=2) as ep,
            tc.tile_pool(name="op", bufs=3) as op,
            tc.tile_pool(name="ps", bufs=3, space=bass.MemorySpace.PSUM) as ps,
        ):
            # sync + scalar are pure DMA-issue queues (no compute on either).
            # One issue per MB keeps the engines essentially idle.
            hw_eng = [nc.sync, nc.scalar]

            # sC is 16 KB — in front of xq0 it delays the fill by <100 ns and
            # the first epilogue (t1 mul) needs it early.
            sC_sb = cst.tile([128, MG, 2], F32)
            nc.sync.dma_start(sC_sb[:], sC[:])

            def load_x(q, eng):
                t = xp.tile([KP, KCH, TQ], X_DT, tag=f"xq{q}")
                eng.dma_start(t[:], xQ[q])
                return t

            def load_w(nb, e, h, eng):
                t = wp.tile([KP, KH, NB], W_DT, tag=f"w{e}_{h}")
                eng.dma_start(t[:], wP[nb, e, h])
                return t

            # Cold-start interleave: the first group's k-loop consumes
            # xq0 + all four W(nb=0) tiles; order them first on the queues.
            x_t = [None] * NQ
            x_t[0] = load_x(0, nc.sync)
            w_cur = {}
            w_cur[0, 0] = load_w(0, 0, 0, nc.scalar)   # pa k0-7
            w_cur[1, 0] = load_w(0, 1, 0, nc.sync)     # pb k0-7
            w_cur[0, 1] = load_w(0, 0, 1, nc.scalar)   # pa k8-15
            w_cur[1, 1] = load_w(0, 1, 1, nc.sync)     # pb k8-15
            # brep (1 MB fp16) queues behind the critical 5 MB fill; it lands
            # right as the first epilogue's bias-add wants it, and the ps
            # pool's 3-group slack absorbs any residual skew.
            brep_sb = cst.tile([128, 2, D_HID], FP16)
            nc.scalar.dma_start(brep_sb[:], brep[:])
            for q in range(1, NQ):
                x_t[q] = load_x(q, hw_eng[q % 2])

            for nb in range(NBLK):
                nb_sl = bass.ts(nb, NB)
                w_next = {} if nb + 1 < NBLK else None
                for q in range(NQ):
                    # W(nb+1) prefetch, one 1 MB tile per even q: spreads the
                    # transfers across the block so no queue burst exceeds
                    # ~3 us, with a full block's lead time.
                    if w_next is not None and q % 2 == 0:
                        i = q // 2
                        e, h = i % 2, i // 2
                        w_next[e, h] = load_w(nb + 1, e, h, hw_eng[i % 2])
                    for mi in range(MPQ):
                        mg = q * MPQ + mi
                        m_sl = bass.ts(mi, 128)
                        pa = ps.tile([128, NB], F32, tag="pa")
                        pb = ps.tile([128, NB], F32, tag="pb")
                        pp = (pa, pb)
                        # half-major order: the first 16 MMs of the very first
                        # group need only wa_h0+wb_h0 (3 MB with xq0), so the
                        # PE starts ~2 us sooner during the cold fill.
                        for h in range(2):
                            for e in range(2):
                                for kk in range(KH):
                                    nc.tensor.matmul(
                                        pp[e][:], x_t[q][:, h * KH + kk, m_sl],
                                        w_cur[e, h][:, kk, :],
                                        start=(h == 0 and kk == 0),
                                        stop=(h == 1 and kk == KH - 1),
                                    )
                        s0 = sC_sb[:, mg, 0:1]
                        s1 = sC_sb[:, mg, 1:2]
                        # epilogue on DVE: out = s0*(pa+bA) + s1*(pb+bB)
                        # (each op reads at most one PSUM input)
                        u = ep.tile([128, NB], F32, tag="u")
                        nc.vector.tensor_add(u[:], pa[:], brep_sb[:, 0, nb_sl])
                        t1 = ep.tile([128, NB], F32, tag="t1")
                        nc.vector.tensor_scalar_mul(t1[:], u[:], s0)
                        v = ep.tile([128, NB], F32, tag="v")
                        nc.vector.tensor_add(v[:], pb[:], brep_sb[:, 1, nb_sl])
                        o = op.tile([128, NB], O_DT, tag="o")
                        nc.vector.scalar_tensor_tensor(
                            o[:], v[:], s1, t1[:], op0=MULT, op1=ADD
                        )
                        # outputs ride the HWDGE queues: the SWDGE (gpsimd)
                        # end-of-kernel drain costs ~6.5 us even when the last
                        # transfer landed long before; HWDGE drains instantly.
                        hw_eng[mg % 2].dma_start(
                            out[bass.ts(mg, 128), nb_sl], o[:]
                        )
                w_cur = w_next

    nc.compile()
    return nc


def _host_gating(x, W_gate, b_gate):
    logits = x @ W_gate + b_gate                       # [N, 8] fp32
    m = logits.max(axis=1, keepdims=True)
    e = np.exp(logits - m)
    gates = e / e.sum(axis=1, keepdims=True)
    idx0 = np.argsort(-gates[0], kind="stable")[:2]    # token-0 top-2 experts
    scores = -np.sort(-gates, axis=1)[:, :2]           # per-token top-2 values
    return idx0, np.ascontiguousarray(scores)


def kernel(x, W_experts, b_experts, W_gate, b_gate):
    global LAST_RESULT
    x = np.ascontiguousarray(np.asarray(x, dtype=np.float32))
    W_experts = np.asarray(W_experts, dtype=np.float32)
    b_experts = np.asarray(b_experts, dtype=np.float32)
    W_gate = np.asarray(W_gate, dtype=np.float32)
    b_gate = np.asarray(b_gate, dtype=np.float32)

    idx0, scores = _host_gating(x, W_gate, b_gate)
    w_np_dt = mybir.dt.np(W_DT)
    x_np_dt = mybir.dt.np(X_DT)
    # wP[nb, e, h, p, kk, n] = W_sel[e][(h*KH+kk)*KP + p, nb*NB + n]
    w_sel = W_experts[idx0].astype(w_np_dt)                        # [2, D_IN, D_HID]
    wP = np.ascontiguousarray(
        w_sel.reshape(2, 2, KH, KP, NBLK, NB)                      # e,h,kk,p,nb,n
        .transpose(4, 0, 1, 3, 2, 5)                               # nb,e,h,p,kk,n
    )
    brep = np.ascontiguousarray(
        np.broadcast_to(b_experts[idx0][None], (128, 2, D_HID))
    ).astype(np.float16)

    xT_full = x.astype(x_np_dt).T                                  # [D_IN, N]

    nc = _build()
    in_maps = []
    for c in range(N_CORES):
        sl = slice(c * NT, (c + 1) * NT)
        # xQ[q, p, k, t] = x[c*NT + q*TQ + t, k*KP + p]
        xQ = np.ascontiguousarray(
            xT_full[:, sl].reshape(KCH, KP, NQ, TQ).transpose(2, 1, 0, 3)
        )
        in_maps.append(
            {
                "xQ": xQ,
                "wP": wP,
                "brep": brep,
                "sC": np.ascontiguousarray(
                    scores[sl].reshape(MG, 128, 2).transpose(1, 0, 2)
                ),
            }
        )

    res = run_bass_kernel_spmd(nc, in_maps, list(range(N_CORES)))
    LAST_RESULT = res
    return np.concatenate(
        [r["out"] for r in res.results], axis=0
    ).astype(np.float32)
